# revision 11
# baseline (speedup 1.0000x reference)
"""CAM module (DANet channel attention) Trainium2 kernel.

Full inputs -> shard batch over 8 cores (2 batches/core) -> single SPMD Bass
kernel (energy + softmax + algebraic BN stats + AllReduce + fused output pass)
-> gather full output.

Math notes:
  energy = q @ q^T per batch; softmax(max-e) == softmax(-e).
  BN stats computed without materializing att@q:
     S1[c] = sum_n M[c,n]   = (att @ qsum)[c]
     S2[c] = sum_n M[c,n]^2 = rowsum((att @ energy) * att)   [M M^T = att E att^T]
  Final: out = s*M + t + q  with s = gamma*bnw*rsd, t = bnb - mean_M*s,
  rsd = 1/sqrt(gamma^2*var_M + eps).

All matmul operands use float32r (fp32 with 12-bit mantissa, 1 cycle/row at
N>=256) -- measured end-to-end error ~3e-4.
"""
import os
import sys

if '/opt/trn_rl_repo' not in sys.path:
    sys.path.insert(0, '/opt/trn_rl_repo')

import numpy as np

import concourse.bass as bass
import concourse.bacc as bacc
import concourse.mybir as mybir
import concourse.tile as tile
from concourse import masks
from concourse.bass_utils import run_bass_kernel_spmd

F32 = mybir.dt.float32
F32R = mybir.dt.float32r

N_CORES = 8
B, C, H, W = 16, 128, 128, 128
HW_FULL = H * W
B_LOC = B // N_CORES
BN_EPS = 1e-5


def build(hw=HW_FULL, n_cores=N_CORES, b_loc=B_LOC, use_collective=True):
    nc = bacc.Bacc("TRN2", target_bir_lowering=False, debug=False,
                   num_devices=n_cores)

    xl = nc.dram_tensor("xl", [b_loc, C, hw], F32, kind="ExternalInput")
    gamma = nc.dram_tensor("gamma", [1, 1], F32, kind="ExternalInput")
    bnw = nc.dram_tensor("bnw", [C, 1], F32, kind="ExternalInput")
    bnb = nc.dram_tensor("bnb", [C, 1], F32, kind="ExternalInput")
    outl = nc.dram_tensor("out", [b_loc, C, hw], F32, kind="ExternalOutput")
    debug = bool(os.environ.get("KERNEL_DEBUG"))
    if debug:
        d_enr = nc.dram_tensor("d_enr", [C, 130], F32, kind="ExternalOutput")
        d_att = nc.dram_tensor("d_att", [C, C], F32, kind="ExternalOutput")
        d_attT = nc.dram_tensor("d_attT", [C, C], F32, kind="ExternalOutput")
        d_r = nc.dram_tensor("d_r", [C, 130], F32, kind="ExternalOutput")
        d_stats = nc.dram_tensor("d_stats", [C, 2], F32, kind="ExternalOutput")
        d_st = nc.dram_tensor("d_st", [C, 2], F32, kind="ExternalOutput")

    n_slices = max(1, hw // 2048)          # DMA/rounding slices
    slice_w = hw // n_slices
    n_groups = hw // 512                   # transpose/matmul groups of 4 chunks
    n_chunks2 = hw // 512                  # pass-2 chunks
    n_total = float(n_cores * b_loc * hw)  # BN sample count per channel

    with tile.TileContext(nc) as tc:
        import contextlib
        ctx = contextlib.ExitStack()
        with ctx:
            singles = ctx.enter_context(tc.tile_pool(name="singles", bufs=1))
            pp_t = ctx.enter_context(tc.tile_pool(name="pp_t", bufs=2, space="PSUM"))
            pp_e = ctx.enter_context(tc.tile_pool(name="pp_e", bufs=2, space="PSUM"))
            pp_s = ctx.enter_context(tc.tile_pool(name="pp_s", bufs=1, space="PSUM"))
            pp_m = ctx.enter_context(tc.tile_pool(name="pp_m", bufs=2, space="PSUM"))
            sm = ctx.enter_context(tc.tile_pool(name="sm", bufs=2))
            pz = ctx.enter_context(tc.tile_pool(name="pz", bufs=3))
            pb = ctx.enter_context(tc.tile_pool(name="pb", bufs=3))
            py = ctx.enter_context(tc.tile_pool(name="py", bufs=3))
            dram = ctx.enter_context(tc.tile_pool(name="dram", bufs=1, space="DRAM"))

            # ---------------- setup ----------------
            ident = singles.tile([128, 128], F32)
            masks.make_identity(nc, ident[:])
            ident_r = singles.tile([128, 128], F32R)
            nc.vector.tensor_copy(ident_r[:], ident[:])

            # staging tiles: per slot [qT(128) | ones(1) | zeros(127)]
            z256 = singles.tile([128, 256], F32)
            nc.vector.memset(z256[:], 0.0)
            nc.vector.memset(z256[:, 128:129], 1.0)
            stages = []
            for i in range(2):
                st = singles.tile([128, 4, 256], F32R, name=f"stage{i}", tag=f"stage{i}")
                for j in range(4):
                    nc.vector.tensor_copy(st[:, j, :], z256[:])
                stages.append(st)

            bnw_sb = singles.tile([128, 1], F32)
            bnb_sb = singles.tile([128, 1], F32)
            nc.sync.dma_start(out=bnw_sb[:], in_=bnw[:, :])
            nc.sync.dma_start(out=bnb_sb[:], in_=bnb[:, :])
            gamma_sb = singles.tile([128, 1], F32)
            g_bcast = bass.AP(tensor=gamma.ap().tensor, offset=0, ap=[[0, 128], [1, 1]])
            nc.sync.dma_start(out=gamma_sb[:], in_=g_bcast)

            q_t = [singles.tile([128, hw], F32R, name=f"q{b}", tag=f"q{b}") for b in range(b_loc)]
            attT_r = [singles.tile([128, 128], F32R, name=f"attT{b}", tag=f"attT{b}") for b in range(b_loc)]
            stats_b = [singles.tile([128, 2], F32, name=f"stats{b}", tag=f"stats{b}") for b in range(b_loc)]

            # ---------------- phase 1: energy + softmax + stats ----------------
            for b in range(b_loc):
                for sl in range(n_slices):
                    cs = slice(sl * slice_w, (sl + 1) * slice_w)
                    bt = pb.tile([128, slice_w], F32, tag="bounce")
                    nc.sync.dma_start(out=bt[:], in_=xl[b, :, cs])
                    # round fp32 -> f32r while copying into the resident q tile
                    nc.vector.tensor_copy(q_t[b][:, cs], bt[:])

                epsum = pp_e.tile([128, 256], F32, tag="epsum")
                for g in range(n_groups):
                    psT = pp_t.tile([128, 4, 128], F32R, tag="psT")
                    for j in range(4):
                        k = 4 * g + j
                        nc.tensor.transpose(
                            psT[:, j, :],
                            q_t[b][:, k * 128:(k + 1) * 128],
                            ident_r[:])
                    stage = stages[g % 2]
                    nc.vector.tensor_copy(stage[:, :, 0:128], psT[:].bitcast(F32))
                    for j in range(4):
                        nc.tensor.matmul(
                            epsum[:],
                            lhsT=stage[:, j, 0:128],
                            rhs=stage[:, j, :],
                            start=(g == 0 and j == 0),
                            stop=(g == n_groups - 1 and j == 3))

                enr = sm.tile([128, 129], F32, tag="enr")
                nc.vector.tensor_copy(enr[:], epsum[:, 0:129])
                # 130 cols: f32r matmul needs an even moving-dim count
                enr_r = sm.tile([128, 130], F32R, tag="enr_r")
                nc.vector.tensor_copy(enr_r[:], epsum[:, 0:130])
                mrow = sm.tile([128, 1], F32, tag="mrow")
                nc.vector.tensor_reduce(out=mrow[:], in_=enr[:, 0:128],
                                        axis=mybir.AxisListType.X,
                                        op=mybir.AluOpType.min)
                p_sb = sm.tile([128, 128], F32, tag="p_sb")
                rs = sm.tile([128, 1], F32, tag="rs")
                nc.scalar.activation(out=p_sb[:], in_=enr[:, 0:128],
                                     func=mybir.ActivationFunctionType.Exp,
                                     bias=mrow[:, 0:1], scale=-1.0,
                                     accum_out=rs[:, 0:1])
                rinv = sm.tile([128, 1], F32, tag="rinv")
                nc.vector.reciprocal(rinv[:], rs[:])
                att_r = sm.tile([128, 128], F32R, tag="att_r")
                nc.vector.tensor_scalar_mul(out=att_r[:], in0=p_sb[:],
                                            scalar1=rinv[:, 0:1])
                attT_ps = pp_s.tile([128, 128], F32R, tag="attT_ps")
                nc.tensor.transpose(attT_ps[:], att_r[:], ident_r[:])
                nc.vector.tensor_copy(attT_r[b][:], attT_ps[:].bitcast(F32))

                r_ps = pp_s.tile([128, 130], F32, tag="r_ps")
                nc.tensor.matmul(r_ps[:], lhsT=attT_r[b][:], rhs=enr_r[:],
                                 start=True, stop=True)
                scr = sm.tile([128, 128], F32, tag="scr")
                nc.vector.tensor_mul(scr[:], r_ps[:, 0:128], att_r[:].bitcast(F32))
                nc.vector.tensor_reduce(out=stats_b[b][:, 1:2], in_=scr[:],
                                        axis=mybir.AxisListType.X,
                                        op=mybir.AluOpType.add)
                nc.vector.tensor_copy(stats_b[b][:, 0:1], r_ps[:, 128:129])
                if debug and b == 0:
                    nc.sync.dma_start(out=d_enr[:, :], in_=enr_r[:].bitcast(F32))
                    nc.sync.dma_start(out=d_att[:, :], in_=att_r[:].bitcast(F32))
                    nc.sync.dma_start(out=d_attT[:, :], in_=attT_r[0][:].bitcast(F32))
                    rcp = sm.tile([128, 130], F32, tag="rcp")
                    nc.vector.tensor_copy(rcp[:], r_ps[:])
                    nc.sync.dma_start(out=d_r[:, :], in_=rcp[:])
                    nc.sync.dma_start(out=d_stats[:, :], in_=stats_b[0][:])

            # ---------------- stats reduce + s,t ----------------
            stats_tot = singles.tile([128, 2], F32)
            nc.vector.tensor_add(stats_tot[:], stats_b[0][:], stats_b[1][:]) \
                if b_loc == 2 else nc.vector.tensor_copy(stats_tot[:], stats_b[0][:])

            if use_collective:
                cc_in = dram.tile([128, 2], F32)
                cc_out = dram.tile([128, 2], F32)
                nc.gpsimd.dma_start(out=cc_in[:], in_=stats_tot[:])
                nc.gpsimd.collective_compute(
                    "AllReduce", mybir.AluOpType.add,
                    replica_groups=[list(range(n_cores))],
                    ins=[cc_in.opt()], outs=[cc_out.opt()])
                stats_g = singles.tile([128, 2], F32)
                nc.gpsimd.dma_start(out=stats_g[:], in_=cc_out[:])
            else:
                stats_g = stats_tot

            inv_n = (1.0 / n_total) if use_collective else (1.0 / (b_loc * hw))
            meanM = singles.tile([128, 1], F32)
            nc.vector.tensor_scalar_mul(out=meanM[:], in0=stats_g[:, 0:1], scalar1=inv_n)
            em2 = singles.tile([128, 1], F32)
            nc.vector.tensor_scalar_mul(out=em2[:], in0=stats_g[:, 1:2], scalar1=inv_n)
            varM = singles.tile([128, 1], F32)
            nc.vector.tensor_mul(varM[:], meanM[:], meanM[:])
            nc.vector.tensor_sub(varM[:], em2[:], varM[:])
            # var_out = gamma^2 * varM ; sd = sqrt(var_out + eps); rsd = 1/sd
            nc.vector.tensor_mul(varM[:], varM[:], gamma_sb[:])
            nc.vector.tensor_mul(varM[:], varM[:], gamma_sb[:])
            eps_sb = singles.tile([128, 1], F32)
            nc.vector.memset(eps_sb[:], BN_EPS)
            sd = singles.tile([128, 1], F32)
            nc.scalar.activation(out=sd[:], in_=varM[:],
                                 func=mybir.ActivationFunctionType.Sqrt,
                                 bias=eps_sb[:, 0:1], scale=1.0)
            rsd = singles.tile([128, 1], F32)
            nc.vector.reciprocal(rsd[:], sd[:])
            s_vec = singles.tile([128, 1], F32)
            nc.vector.tensor_mul(s_vec[:], gamma_sb[:], bnw_sb[:])
            nc.vector.tensor_mul(s_vec[:], s_vec[:], rsd[:])
            t_vec = singles.tile([128, 1], F32)
            nc.vector.tensor_mul(t_vec[:], meanM[:], s_vec[:])
            nc.vector.tensor_sub(t_vec[:], bnb_sb[:], t_vec[:])
            if debug:
                stv = singles.tile([128, 2], F32)
                nc.vector.tensor_copy(stv[:, 0:1], s_vec[:])
                nc.vector.tensor_copy(stv[:, 1:2], t_vec[:])
                nc.sync.dma_start(out=d_st[:, :], in_=stv[:])

            # ---------------- phase 2: out = s*(att@q) + t + q ----------------
            for b in range(b_loc):
                for c in range(n_chunks2):
                    cs = slice(c * 512, (c + 1) * 512)
                    mps = pp_m.tile([128, 512], F32, tag="mps")
                    nc.tensor.matmul(mps[:], lhsT=attT_r[b][:],
                                     rhs=q_t[b][:, cs],
                                     start=True, stop=True)
                    z = pz.tile([128, 512], F32, tag="z")
                    nc.scalar.activation(out=z[:], in_=mps[:],
                                         func=mybir.ActivationFunctionType.Identity,
                                         bias=t_vec[:, 0:1], scale=s_vec[:, 0:1])
                    y = py.tile([128, 512], F32, tag="y")
                    nc.vector.tensor_add(y[:], z[:], q_t[b][:, cs].bitcast(F32))
                    nc.sync.dma_start(out=outl[b, :, cs], in_=y[:])

    nc.compile()
    return nc


_CACHE = {}


def _get_nc(hw=HW_FULL):
    if hw not in _CACHE:
        _CACHE[hw] = build(hw=hw,
                           use_collective=not os.environ.get("KERNEL_NO_CC"))
    return _CACHE[hw]


def kernel(x, gamma, bn_weight, bn_bias):
    x = np.ascontiguousarray(np.asarray(x, dtype=np.float32))
    gamma = np.asarray(gamma, dtype=np.float32).reshape(1, 1)
    bnw = np.ascontiguousarray(np.asarray(bn_weight, dtype=np.float32).reshape(C, 1))
    bnb = np.ascontiguousarray(np.asarray(bn_bias, dtype=np.float32).reshape(C, 1))
    Bf, Cf, Hf, Wf = x.shape
    hw = Hf * Wf
    xr = x.reshape(Bf, Cf, hw)

    nc = _get_nc(hw)
    in_maps = []
    for i in range(N_CORES):
        in_maps.append({
            "xl": np.ascontiguousarray(xr[i * B_LOC:(i + 1) * B_LOC]),
            "gamma": gamma, "bnw": bnw, "bnb": bnb,
        })
    res = run_bass_kernel_spmd(nc, in_maps, core_ids=list(range(N_CORES)))
    out = np.concatenate([r["out"] for r in res.results], axis=0)
    return out.reshape(Bf, Cf, Hf, Wf).astype(np.float32)


if __name__ == "__main__":
    rng = np.random.default_rng(0)
    x = rng.standard_normal((B, C, H, W), dtype=np.float32)
    g = rng.standard_normal((1,), dtype=np.float32)
    w = rng.random((C,), dtype=np.float32)
    bchan = rng.standard_normal((C,), dtype=np.float32)
    out = kernel(x, g, w, bchan)
    print("kernel ran, out shape", out.shape)
